# revision 1
# baseline (speedup 1.0000x reference)
"""Transformer block (dense_transformer) on 8 TRN2 NeuronCores.

Strategy: pure data-parallel over batch (B=128 -> 16 items/core), weights
replicated. Per item, all linear layers run feature-major ([feat, T] with
feat on partitions) so matmul outputs land directly in the layout the next
matmul consumes; LayerNorm/softmax run token-major ([T, feat]) where their
reductions are free-axis reductions. PE transposes convert between the two.
Matmul operands use float32r (TF32-like, ~1 cyc/row at N>=256) — final
output error vs the fp32 reference is ~1e-4 absmax because matmul error only
perturbs the residual corrections.
"""

import os

import numpy as np

import concourse.bass as bass
import concourse.mybir as mybir
from concourse.tile import TileContext
from concourse.vector_clock import ScopedClock

F32 = mybir.dt.float32
F32R = mybir.dt.float32r
AF = mybir.ActivationFunctionType
AX = mybir.AxisListType
ALU = mybir.AluOpType

B, T, C, H, D = 128, 256, 384, 6, 64
F = 4 * C
NCORES = 8
BL = B // NCORES
P = 128
TT = T // P    # 2 token tiles
CT = C // P    # 3 channel tiles
FT = F // P    # 12 ffn-hidden tiles
LN_EPS = 1e-5
CSCALE = float(C) ** -0.5
NEG = -1.0e9
_STOP = int(os.environ.get("KSTOP", "99"))


class PatchedTileContext(TileContext):
    """Workaround for this container's walrus: BIR instructions may carry at
    most ONE attached sem wait. Hoist extras into standalone waits."""

    def _hoist_multi_waits(self):
        nc = self.nc
        assert self.sems is not None
        sem_by_num = {s.num: s for s in self.sems.allocated().values()}
        for func in nc.m.functions:
            for blk in func.blocks:
                insts = blk.instructions
                i = 0
                while i < len(insts):
                    inst = insts[i]
                    si = inst.sync_info
                    waits = list(si.on_wait) if (si and si.on_wait) else []
                    if len(waits) <= 1:
                        i += 1
                        continue
                    hoist = waits[1:]
                    for w in hoist:
                        if not (
                            w.sync_type == "semaphore"
                            and w.wait_mode == "sem-ge-imm"
                            and w.id in sem_by_num
                        ):
                            raise RuntimeError(
                                f"cannot hoist waits on {inst.name}: {waits}"
                            )
                    del si.on_wait[1:]
                    engine = nc.engines[inst.engine]
                    new_insts = []
                    for w in hoist:
                        wi = engine.wait_ge(sem_by_num[w.id], w.wait_value)
                        new_insts.append(wi.ins)
                    cur_list = nc.cur_bb.bb.instructions
                    for ni in new_insts:
                        cur_list.remove(ni)
                    insts[i:i] = new_insts
                    i += len(new_insts) + 1

    def _drain_and_barrier(self, tick_clock, wait_clock):
        nc = self.nc
        self._hoist_multi_waits()

        drain_inst = nc.sync.drain()
        wait_clock.add_sem_waits(
            drain_inst.ins, ScopedClock({None: tick_clock.global_clock})
        )
        waits = list(drain_inst.ins.sync_info.on_wait or [])
        if len(waits) > 1:
            drain_inst.ins.sync_info.on_wait.clear()
            assert self.sems is not None
            sem_by_num = {s.num: s for s in self.sems.allocated().values()}
            new_waits = []
            for w in waits:
                assert w.sync_type == "semaphore" and w.wait_mode == "sem-ge-imm", w
                new_waits.append(nc.sync.wait_ge(sem_by_num[w.id], w.wait_value))
            bb = nc.cur_bb.bb
            insts = bb.instructions
            names = [i.name for i in insts]
            di = names.index(drain_inst.ins.name)
            tail = insts[di + 1 : di + 1 + len(new_waits)]
            assert len(tail) == len(new_waits)
            insts[di : di + 1 + len(new_waits)] = tail + [drain_inst.ins]

        nc.all_engine_barrier()
        assert self.sems is not None
        popped = nc._tile_sem_poison_stack.pop()
        assert popped is self._sem_poison
        nc.clear_and_free_semaphores(list(self.sems.allocated().values()))
        nc.all_engine_barrier()


def ts(i, n=P):
    return slice(i * n, (i + 1) * n)


def build_nc():
    nc = bass.Bass()
    x_in = nc.dram_tensor("x", [BL, T, C], F32, kind="ExternalInput")
    wq_in = nc.dram_tensor("wqf", [C, C], F32, kind="ExternalInput")
    wk_in = nc.dram_tensor("wkf", [C, C], F32, kind="ExternalInput")
    wv_in = nc.dram_tensor("wvf", [C, C], F32, kind="ExternalInput")
    wp_in = nc.dram_tensor("wpf", [C, C], F32, kind="ExternalInput")
    w1_in = nc.dram_tensor("w1f", [C, F], F32, kind="ExternalInput")
    w2_in = nc.dram_tensor("w2f", [F, C], F32, kind="ExternalInput")
    gb_in = nc.dram_tensor("gb", [6, C], F32, kind="ExternalInput")
    b1_in = nc.dram_tensor("b1v", [F], F32, kind="ExternalInput")
    id_in = nc.dram_tensor("ident", [P, P], F32, kind="ExternalInput")
    m_in = nc.dram_tensor("masks", [TT, P, T], F32, kind="ExternalInput")
    out_t = nc.dram_tensor("out", [BL, T, C], F32, kind="ExternalOutput")

    IP = 2               # items per group
    NG = BL // IP        # groups
    W = IP * T           # moving width for feature-major matmuls (512)

    with PatchedTileContext(nc) as tc:
        with tc.tile_pool(name="consts", bufs=1) as consts:
            with tc.tile_pool(name="wstage", bufs=2) as wload:
                def load_f32r(ap_dram, kt, m, tag):
                    stage = wload.tile([P, kt, m], F32, tag="wstage")
                    nc.sync.dma_start(
                        stage[:], ap_dram.rearrange("(kt p) m -> p kt m", p=P)
                    )
                    wr = consts.tile([P, kt, m], F32R, tag=tag)
                    nc.vector.tensor_copy(wr[:], stage[:])
                    return wr

                wq_r = load_f32r(wq_in[:], CT, C, "wq")
                wk_r = load_f32r(wk_in[:], CT, C, "wk")
                wv_r = load_f32r(wv_in[:], CT, C, "wv")
                wp_r = load_f32r(wp_in[:], CT, C, "wp")
                w1_r = load_f32r(w1_in[:], CT, F, "w1")
                w2_r = load_f32r(w2_in[:], FT, C, "w2")

                id_f = consts.tile([P, P], F32, tag="idf")
                nc.sync.dma_start(id_f[:], id_in[:])
                id_r = consts.tile([P, P], F32R, tag="idr")
                nc.vector.tensor_copy(id_r[:], id_f[:])

                mstage = wload.tile([P, TT, T], F32, tag="wstage")
                nc.sync.dma_start(mstage[:], m_in.rearrange("tt p t -> p tt t"))
                m_r = consts.tile([P, TT, T], F32R, tag="mr")
                nc.vector.tensor_copy(m_r[:], mstage[:])

                gb = consts.tile([P, 6, CT], F32, tag="gb")
                nc.sync.dma_start(gb[:], gb_in.rearrange("g (ct p) -> p g ct", p=P))
                b1c = consts.tile([P, FT], F32, tag="b1c")
                nc.sync.dma_start(b1c[:], b1_in.rearrange("(ft p) -> p ft", p=P))
                epsc = consts.tile([P, 1], F32, tag="eps")
                nc.gpsimd.memset(epsc[:], LN_EPS)

            g1c = gb[:, 0, :]
            be1c = gb[:, 1, :]
            g2c = gb[:, 2, :]
            be2c = gb[:, 3, :]
            bpc = gb[:, 4, :]
            b2c = gb[:, 5, :]

            with (
                tc.tile_pool(name="act", bufs=2) as actp,
                tc.tile_pool(name="xn", bufs=3) as xnp,
                tc.tile_pool(name="fm", bufs=2) as fmp,
                tc.tile_pool(name="zp", bufs=1) as zp,
                tc.tile_pool(name="attn", bufs=4) as attnp,
                tc.tile_pool(name="stats", bufs=8) as stats,
                tc.tile_pool(name="ps5", bufs=4, space="PSUM") as ps5,
                tc.tile_pool(name="ps2", bufs=4, space="PSUM") as ps2,
            ):
                def load_x(g):
                    xt = actp.tile([P, IP, TT, C], F32, tag="x")
                    nc.sync.dma_start(
                        xt[:],
                        x_in[g * IP : (g + 1) * IP].rearrange(
                            "i (tt p) c -> p i tt c", p=P
                        ),
                    )
                    return xt

                def ln_stats_xn(src):
                    """src [P, IP, TT, C] -> xn [P, 4, C] f32r normalized
                    (no affine -- g/be folded into the ln_fm copies)."""
                    s4 = src.rearrange("p i tt c -> p (i tt) c")
                    nseg = IP * TT
                    ss = stats.tile([P, 2, nseg], F32, tag="ss")
                    nc.vector.reduce_sum(ss[:, 0, :], s4, axis=AX.X)
                    sq = actp.tile([P, C], F32, tag="sq")
                    for seg in range(nseg):
                        nc.scalar.activation(
                            sq[:], s4[:, seg, :], AF.Square,
                            accum_out=ss[:, 1, seg : seg + 1],
                        )
                    mue = stats.tile([P, 2, nseg], F32, tag="mue")
                    nc.scalar.mul(mue[:], ss[:], 1.0 / C)
                    mu2 = stats.tile([P, nseg], F32, tag="mu2")
                    nc.scalar.activation(mu2[:], mue[:, 0, :], AF.Square)
                    var = stats.tile([P, nseg], F32, tag="var")
                    nc.vector.tensor_tensor(var[:], mue[:, 1, :], mu2[:], ALU.subtract)
                    lnv = stats.tile([P, nseg], F32, tag="lnv")
                    nc.scalar.activation(lnv[:], var[:], AF.Ln, bias=epsc[:])
                    rstd = stats.tile([P, nseg], F32, tag="rstd")
                    nc.scalar.activation(rstd[:], lnv[:], AF.Exp, scale=-0.5)
                    xn = xnp.tile([P, nseg, C], F32R, tag="xn")
                    for seg in range(nseg):
                        nc.vector.tensor_scalar(
                            xn[:, seg, :], s4[:, seg, :],
                            mue[:, 0, seg : seg + 1],
                            rstd[:, seg : seg + 1],
                            ALU.subtract, ALU.mult,
                        )
                    return xn

                def ln_fm(xn, gcol, becol):
                    """xn [P, 4, C] -> h_ct [P, CT, IP, T] f32r with affine."""
                    h_ct = fmp.tile([P, CT, IP, T], F32R, tag="hct")
                    for ct in range(CT):
                        ps = ps5.tile([P, IP, T], F32R, tag="ps5")
                        for i in range(IP):
                            for tt in range(TT):
                                nc.tensor.transpose(
                                    ps[:, i, ts(tt)],
                                    xn[:, i * TT + tt, ts(ct)],
                                    id_r[:],
                                )
                        nc.scalar.activation(
                            h_ct[:, ct, :, :], ps[:], AF.Identity,
                            bias=becol[:, ct : ct + 1],
                            scale=gcol[:, ct : ct + 1],
                        )
                        yield
                    return h_ct

                def front(g, x_t):
                    xn1 = ln_stats_xn(x_t[:])
                    yield
                    h_ct = yield from ln_fm(xn1, g1c, be1c)

                    qT = fmp.tile([P, CT, IP, T], F32R, tag="fmA")
                    kT = fmp.tile([P, CT, IP, T], F32R, tag="fmB")
                    for m in range(CT):
                        psq = ps5.tile([P, IP, T], F32, tag="ps5")
                        psk = ps5.tile([P, IP, T], F32, tag="ps5")
                        for k in range(CT):
                            nc.tensor.matmul(
                                psq[:], wq_r[:, k, ts(m)], h_ct[:, k, :, :],
                                start=(k == 0), stop=(k == CT - 1),
                            )
                            nc.tensor.matmul(
                                psk[:], wk_r[:, k, ts(m)], h_ct[:, k, :, :],
                                start=(k == 0), stop=(k == CT - 1),
                            )
                        nc.scalar.copy(qT[:, m, :, :], psq[:])
                        nc.vector.tensor_copy(kT[:, m, :, :], psk[:])
                        yield
                    v_sb = fmp.tile([P, IP, TT, C], F32R, tag="fmC")
                    for i in range(IP):
                        for st in range(TT):
                            psv = ps2.tile([P, C], F32, tag="ps2")
                            for k in range(CT):
                                nc.tensor.matmul(
                                    psv[:], h_ct[:, k, i, ts(st)], wv_r[:, k, :],
                                    start=(k == 0), stop=(k == CT - 1),
                                )
                            if st == 0:
                                nc.scalar.copy(v_sb[:, i, st, :], psv[:])
                            else:
                                nc.vector.tensor_copy(v_sb[:, i, st, :], psv[:])
                            yield

                    attnT = fmp.tile([P, CT, IP, T], F32R, tag="fmC")
                    for i in range(IP):
                        for h in range(H):
                            j, off = h // 2, (h % 2) * 64
                            w_n = attnp.tile([P, TT, T], F32R, tag="wn")
                            for tt in range(TT):
                                pss = ps2.tile([P, T], F32, tag="ps2")
                                nc.tensor.matmul(
                                    pss[:],
                                    qT[off : off + 64, j, i, ts(tt)],
                                    kT[off : off + 64, j, i, :],
                                    start=True, stop=False,
                                )
                                nc.tensor.matmul(
                                    pss[:], id_r[:], m_r[:, tt, :],
                                    start=False, stop=True,
                                )
                                lim = P if tt == 0 else T
                                we = attnp.tile([P, T], F32, tag="we")
                                rowsum = stats.tile([P, 1], F32, tag="rs")
                                nc.scalar.activation(
                                    we[:, :lim], pss[:, :lim], AF.Exp,
                                    scale=CSCALE, accum_out=rowsum[:],
                                )
                                rec = stats.tile([P, 1], F32, tag="rec")
                                nc.vector.reciprocal(rec[:], rowsum[:])
                                nc.vector.tensor_scalar_mul(
                                    w_n[:, tt, :lim], we[:, :lim], rec[:]
                                )
                            wT = attnp.tile([P, TT, T], F32R, tag="wT")
                            psw = ps2.tile([P, T], F32R, tag="ps2")
                            for tt in range(TT):
                                nc.tensor.transpose(
                                    psw[:, ts(tt)], w_n[:, tt, ts(0)], id_r[:]
                                )
                            nc.vector.tensor_copy(wT[:, 0, :], psw[:])
                            psw1 = ps2.tile([P, T], F32R, tag="ps2")
                            nc.tensor.transpose(
                                psw1[:, ts(1)], w_n[:, 1, ts(1)], id_r[:]
                            )
                            nc.vector.tensor_copy(
                                wT[:, 1, ts(1)], psw1[:, ts(1)]
                            )
                            psa = ps2.tile([64, T], F32, tag="ps2")
                            nc.tensor.matmul(
                                psa[:],
                                v_sb[:, i, 0, h * 64 : (h + 1) * 64],
                                wT[:, 0, :],
                                start=True, stop=False,
                            )
                            nc.tensor.matmul(
                                psa[:, ts(1)],
                                v_sb[:, i, 1, h * 64 : (h + 1) * 64],
                                wT[:, 1, ts(1)],
                                start=False, stop=True,
                            )
                            nc.vector.tensor_copy(
                                attnT[off : off + 64, j, i, :], psa[:]
                            )
                            yield

                    saT = fmp.tile([P, CT, IP, T], F32R, tag="fmA")
                    for m in range(CT):
                        psj = ps5.tile([P, IP, T], F32, tag="ps5")
                        for k in range(CT):
                            nc.tensor.matmul(
                                psj[:], wp_r[:, k, ts(m)], attnT[:, k, :, :],
                                start=(k == 0), stop=(k == CT - 1),
                            )
                        nc.scalar.activation(
                            saT[:, m, :, :], psj[:], AF.Identity,
                            bias=bpc[:, m : m + 1],
                        )
                        yield
                    x1 = actp.tile([P, IP, TT, C], F32, tag="x1")
                    for i in range(IP):
                        for tt in range(TT):
                            psr = ps2.tile([P, C], F32R, tag="ps2")
                            for ct in range(CT):
                                nc.tensor.transpose(
                                    psr[:, ts(ct)], saT[:, ct, i, ts(tt)], id_r[:]
                                )
                            nc.vector.tensor_tensor(
                                x1[:, i, tt, :], psr[:], x_t[:, i, tt, :], ALU.add
                            )
                            yield
                    xn2 = ln_stats_xn(x1[:])
                    return x1, xn2

                def back(g, x1, xn2):
                    h2_ct = yield from ln_fm(xn2, g2c, be2c)
                    z = zp.tile([P, FT, IP, T], F32R, tag="z")
                    for m in range(FT):
                        psz = ps5.tile([P, IP, T], F32, tag="ps5")
                        for k in range(CT):
                            nc.tensor.matmul(
                                psz[:], w1_r[:, k, ts(m)], h2_ct[:, k, :, :],
                                start=(k == 0), stop=(k == CT - 1),
                            )
                        nc.scalar.activation(
                            z[:, m, :, :], psz[:], AF.Relu,
                            bias=b1c[:, m : m + 1],
                        )
                        yield
                    yT = fmp.tile([P, CT, IP, T], F32R, tag="fmB")
                    for m in range(CT):
                        psy = ps5.tile([P, IP, T], F32, tag="ps5")
                        for k in range(FT):
                            nc.tensor.matmul(
                                psy[:], w2_r[:, k, ts(m)], z[:, k, :, :],
                                start=(k == 0), stop=(k == FT - 1),
                            )
                        nc.scalar.activation(
                            yT[:, m, :, :], psy[:], AF.Identity,
                            bias=b2c[:, m : m + 1],
                        )
                        yield
                    for i in range(IP):
                        for tt in range(TT):
                            pso = ps2.tile([P, C], F32R, tag="ps2")
                            for ct in range(CT):
                                nc.tensor.transpose(
                                    pso[:, ts(ct)], yT[:, ct, i, ts(tt)], id_r[:]
                                )
                            o = actp.tile([P, C], F32, tag="o")
                            nc.vector.tensor_tensor(
                                o[:], pso[:], x1[:, i, tt, :], ALU.add
                            )
                            nc.sync.dma_start(out_t[g * IP + i, ts(tt), :], o[:])
                            yield

                def run_all(gens):
                    """Round-robin the generators; return list of returns."""
                    rets = {}
                    live = {id(gn): gn for gn in gens}
                    order = [id(gn) for gn in gens]
                    while live:
                        for key in list(order):
                            gn = live.get(key)
                            if gn is None:
                                continue
                            try:
                                next(gn)
                            except StopIteration as e:
                                rets[key] = e.value
                                del live[key]
                    return [rets[id(gn)] for gn in gens]

                x_next = load_x(0)
                pending = None
                for g in range(NG):
                    x_t = x_next
                    if g + 1 < NG:
                        x_next = load_x(g + 1)
                    gens = [front(g, x_t)]
                    if pending is not None:
                        gens.append(back(*pending))
                    rets = run_all(gens)
                    pending = (g,) + rets[0]
                run_all([back(*pending)])
    return nc


_NC_CACHE = None


def _get_nc():
    global _NC_CACHE
    if _NC_CACHE is None:
        _NC_CACHE = build_nc()
    return _NC_CACHE


def _host_consts():
    ident = np.eye(P, dtype=np.float32)
    masks = np.zeros((TT, P, T), dtype=np.float32)
    for tt in range(TT):
        trow = np.arange(P) + tt * P
        scol = np.arange(T)
        masks[tt][scol[None, :] > trow[:, None]] = NEG
    return ident, masks


def kernel(x, Wq, Wk, Wv, Wp, bp, W1, b1, W2, b2, g1, be1, g2, be2):
    x = np.ascontiguousarray(np.asarray(x, np.float32))
    WqF = np.ascontiguousarray(
        np.asarray(Wq, np.float32).transpose(1, 0, 2).reshape(C, C)
    )
    WkF = np.ascontiguousarray(
        np.asarray(Wk, np.float32).transpose(1, 0, 2).reshape(C, C)
    )
    WvF = np.ascontiguousarray(
        np.asarray(Wv, np.float32).transpose(1, 0, 2).reshape(C, C)
    )
    WpF = np.ascontiguousarray(np.asarray(Wp, np.float32))
    W1F = np.ascontiguousarray(np.asarray(W1, np.float32))
    W2F = np.ascontiguousarray(np.asarray(W2, np.float32))
    gb = np.ascontiguousarray(
        np.stack([
            np.asarray(g1, np.float32), np.asarray(be1, np.float32),
            np.asarray(g2, np.float32), np.asarray(be2, np.float32),
            np.asarray(bp, np.float32), np.asarray(b2, np.float32),
        ])
    )
    b1v = np.ascontiguousarray(np.asarray(b1, np.float32))
    ident, masks = _host_consts()

    nc = _get_nc()
    shared = {
        "wqf": WqF, "wkf": WkF, "wvf": WvF, "wpf": WpF,
        "w1f": W1F, "w2f": W2F, "gb": gb, "b1v": b1v,
        "ident": ident, "masks": masks,
    }
    in_maps = []
    for c in range(NCORES):
        m = dict(shared)
        m["x"] = np.ascontiguousarray(x[c * BL : (c + 1) * BL])
        in_maps.append(m)

    from concourse.bass_utils import run_bass_kernel_spmd

    res = run_bass_kernel_spmd(nc, in_maps, list(range(NCORES)))
    out = np.concatenate([res.results[c]["out"] for c in range(NCORES)], axis=0)
    return out.astype(np.float32)

